# revision 1
# baseline (speedup 1.0000x reference)
"""Block-diagonal 2x2 equalizer kernel for Trainium2 (8 NeuronCores).

Per point (b, u, s, f) solves the 2x2 system M x = v by Cramer's rule:
    m_ij = h[b, pi[u], i, 0, 2u+j, s, f]   (only 1/4 of h is needed)
    det  = m00*m11 - m01*m10
    x0   = (m11*v0 - m01*v1) / det
    x1   = (m00*v1 - m10*v0) / det
    out[b, u, a, s, f] = x_a

Sharding: data-parallel over batch, 2 batches per core on 8 cores. The host
gathers (precoding_ind) and packs operand planes into contiguous [128, fd]
blocks so every device DMA is a large fully-contiguous transfer.

Device kernel is raw Bass (no TileContext): the neuronxcc walrus used by the
axon/bass2jax path allows only one sync-wait per instruction, so all waits
are standalone wait_ge instructions and every SBUF buffer is written exactly
once (pure dataflow, per-chunk semaphores, no WAR hazards, no tail barrier).

Pipeline (NCH chunks over the u axis):
  sync engine:  per chunk, loads A={m00|m11}, B={m01|m10}, Y={v0|v1}
  DVE:          all 11 tensor ops per chunk (p0, p1, det, q0, q1, r0, q2,
                q3, r1, x0, x1). GPSIMD is intentionally UNUSED: measured
                on HW, concurrent GPSIMD+DVE contend for SBUF ports and
                drop combined throughput below DVE alone (DVE TT 1.09us
                -> 2.9us while GPSIMD runs).
  ACT (scalar): rdet = Reciprocal(det) via direct InstActivation (HW
                spline measured 2.2e-5 max rel err, 1.04us vs 5.75us for
                DVE reciprocal at FD=896); also issues the stores
"""

from contextlib import ExitStack

import numpy as np

import concourse.bass as bass
import concourse.mybir as mybir
from concourse.bass_utils import run_bass_kernel_spmd

# Problem shapes (hardcoded per contract)
B, U, A, NTX, T, S, F = 16, 4, 2, 1, 8, 14, 2048
SF = S * F               # 28672
NCORES = 8
BPC = B // NCORES        # 2 batches per core
NCH = 2                  # pipeline chunks (groups of u)
UPC = U // NCH           # u's per chunk
QW = 448                 # inner width: SF = 64 * 448
ROWS = SF // QW          # 64 rows -> partition p = b*64 + row
FD = UPC * QW            # free elems per component per chunk

# Set by test harness to capture an NTFF profile on the run.
TRACE = False
LAST_RESULTS = None


def _pack(d):
    """[BPC, U, SF] -> [NCH, 128, FD] with p = b*ROWS + sf//QW, f = ul*QW + sf%QW."""
    d = d.reshape(BPC, U, ROWS, QW)
    out = np.empty((NCH, BPC * ROWS, FD), np.float32)
    for k in range(NCH):
        blk = d[:, k * UPC:(k + 1) * UPC]               # [BPC, UPC, ROWS, QW]
        out[k] = blk.transpose(0, 2, 1, 3).reshape(BPC * ROWS, FD)
    return out


def _unpack(t):
    """Inverse of _pack: [NCH, 128, FD] -> [BPC, U, SF]."""
    out = np.empty((BPC, U, ROWS, QW), np.float32)
    for k in range(NCH):
        blk = t[k].reshape(BPC, ROWS, UPC, QW).transpose(0, 2, 1, 3)
        out[:, k * UPC:(k + 1) * UPC] = blk
    return out.reshape(BPC, U, SF)


def _build_nc():
    f32 = mybir.dt.float32
    nc = bass.Bass("TRN2")
    # hA: [m00 | m11], hB: [m01 | m10], yB: [v0 | v1], xout: [x0 | x1]
    hA = nc.dram_tensor("hA", [NCH, 128, 2 * FD], f32, kind="ExternalInput")
    hB = nc.dram_tensor("hB", [NCH, 128, 2 * FD], f32, kind="ExternalInput")
    yB = nc.dram_tensor("yB", [NCH, 128, 2 * FD], f32, kind="ExternalInput")
    xout = nc.dram_tensor("xout", [NCH, 128, 2 * FD], f32, kind="ExternalOutput")

    with ExitStack() as ctx:
        tA = [ctx.enter_context(nc.sbuf_tensor(f"tA{k}", [128, 2 * FD], f32)) for k in range(NCH)]
        tB = [ctx.enter_context(nc.sbuf_tensor(f"tB{k}", [128, 2 * FD], f32)) for k in range(NCH)]
        tY = [ctx.enter_context(nc.sbuf_tensor(f"tY{k}", [128, 2 * FD], f32)) for k in range(NCH)]
        tX = [ctx.enter_context(nc.sbuf_tensor(f"tX{k}", [128, 2 * FD], f32)) for k in range(NCH)]
        tp = [
            {
                n: ctx.enter_context(nc.sbuf_tensor(f"{n}_{k}", [128, FD], f32))
                for n in ("p0", "p1", "q2", "q3", "det", "rdet", "q0", "q1", "r0", "r1")
            }
            for k in range(NCH)
        ]
        semA = [ctx.enter_context(nc.semaphore(f"semA{k}")) for k in range(NCH)]
        semB = [ctx.enter_context(nc.semaphore(f"semB{k}")) for k in range(NCH)]
        semY = [ctx.enter_context(nc.semaphore(f"semY{k}")) for k in range(NCH)]
        semO = [ctx.enter_context(nc.semaphore(f"semO{k}")) for k in range(NCH)]
        dve_sem = ctx.enter_context(nc.semaphore("dve_sem"))
        act_sem = ctx.enter_context(nc.semaphore("act_sem"))

        with nc.Block() as block:

            @block.sync
            def _(sync):
                for k in range(NCH):
                    sync.dma_start(out=tA[k][:], in_=hA[k]).then_inc(semA[k], 16)
                    sync.dma_start(out=tY[k][:], in_=yB[k]).then_inc(semY[k], 16)
                    sync.dma_start(out=tB[k][:], in_=hB[k]).then_inc(semB[k], 16)

            # dve_sem counts: chunk k ops are 11k+1 .. 11k+11
            # order chosen so every consumer is >=2 ops after its producers
            # (a wait_ge on a just-finished DVE op stalls ~1-2us for the
            # producer's pipe DRAIN + sem propagation; with distance the
            # waits are already satisfied): p0 q0 q2 p1 q1 q3 det r0 r1 x0 x1
            @block.vector
            def _(vector):
                for k in range(NCH):
                    a, b, y, x, t = tA[k], tB[k], tY[k], tX[k], tp[k]
                    m00, m11 = a[:, :FD], a[:, FD:]
                    m01, m10 = b[:, :FD], b[:, FD:]
                    v0, v1 = y[:, :FD], y[:, FD:]
                    c = 11 * k
                    vector.wait_ge(semA[k], 16)
                    vector.tensor_mul(t["p0"][:], m00, m11).then_inc(dve_sem, 1)   # c+1
                    vector.wait_ge(semY[k], 16)
                    vector.tensor_mul(t["q0"][:], m11, v0).then_inc(dve_sem, 1)    # c+2
                    vector.tensor_mul(t["q2"][:], m00, v1).then_inc(dve_sem, 1)    # c+3
                    vector.wait_ge(semB[k], 16)
                    vector.tensor_mul(t["p1"][:], m01, m10).then_inc(dve_sem, 1)   # c+4
                    vector.tensor_mul(t["q1"][:], m01, v1).then_inc(dve_sem, 1)    # c+5
                    vector.tensor_mul(t["q3"][:], m10, v0).then_inc(dve_sem, 1)    # c+6
                    vector.wait_ge(dve_sem, c + 4)
                    vector.tensor_sub(t["det"][:], t["p0"][:], t["p1"][:]).then_inc(
                        dve_sem, 1
                    )  # c+7  (ACT recip consumes)
                    vector.wait_ge(dve_sem, c + 5)
                    vector.tensor_sub(t["r0"][:], t["q0"][:], t["q1"][:]).then_inc(
                        dve_sem, 1
                    )  # c+8
                    vector.wait_ge(dve_sem, c + 6)
                    vector.tensor_sub(t["r1"][:], t["q2"][:], t["q3"][:]).then_inc(
                        dve_sem, 1
                    )  # c+9
                    vector.wait_ge(dve_sem, c + 8)
                    vector.wait_ge(act_sem, k + 1)
                    vector.tensor_mul(x[:, :FD], t["r0"][:], t["rdet"][:]).then_inc(
                        dve_sem, 1
                    )  # c+10 (store x0 consumes)
                    vector.wait_ge(dve_sem, c + 9)
                    vector.tensor_mul(x[:, FD:], t["r1"][:], t["rdet"][:]).then_inc(
                        dve_sem, 1
                    )  # c+11 (store x1 consumes)

            @block.scalar
            def _(scalar):
                for k in range(NCH):
                    c = 11 * k
                    scalar.wait_ge(dve_sem, c + 7)
                    scalar.add_instruction(
                        mybir.InstActivation(
                            name=nc.get_next_instruction_name(),
                            func=mybir.ActivationFunctionType.Reciprocal,
                            ins=[
                                scalar.lower_ap(tp[k]["det"][:]),
                                mybir.ImmediateValue(dtype=f32, value=0.0),
                                mybir.ImmediateValue(dtype=f32, value=1.0),
                                mybir.ImmediateValue(dtype=f32, value=0.0),
                            ],
                            outs=[scalar.lower_ap(tp[k]["rdet"][:])],
                        )
                    ).then_inc(act_sem, 1)
                    scalar.wait_ge(dve_sem, c + 10)
                    scalar.dma_start(out=xout[k, :, :FD], in_=tX[k][:, :FD]).then_inc(
                        semO[k], 16
                    )
                    scalar.wait_ge(dve_sem, c + 11)
                    scalar.dma_start(out=xout[k, :, FD:], in_=tX[k][:, FD:]).then_inc(
                        semO[k], 16
                    )
                for k in range(NCH):
                    scalar.wait_ge(semO[k], 32)

    return nc


def make_in_maps(y, h, precoding_ind):
    """Host-side gather + pack. Returns per-core input maps."""
    y = np.asarray(y)
    h = np.asarray(h)
    pi = np.asarray(precoding_ind).astype(np.int64)

    hg = h[:, pi[0]]                                     # [B, U, A, NTX, T, S, F]
    # hsel[b, u, i, j] = hg[b, u, i, 0, 2u+j]  -> components c = i*2+j
    hsel = np.stack(
        [hg[:, u, :, 0, 2 * u:2 * u + 2] for u in range(U)], axis=1
    )                                                    # [B, U, A(i), 2(j), S, F]
    hsel = np.ascontiguousarray(hsel).reshape(B, U, 4, SF).astype(np.float32)
    yr = np.ascontiguousarray(y).reshape(B, U, A, SF).astype(np.float32)

    in_maps = []
    for c in range(NCORES):
        b0 = c * BPC
        hs = hsel[b0:b0 + BPC]                           # [BPC, U, 4, SF]
        ys = yr[b0:b0 + BPC]                             # [BPC, U, A, SF]
        hA = np.concatenate([_pack(hs[:, :, 0]), _pack(hs[:, :, 3])], axis=2)
        hB = np.concatenate([_pack(hs[:, :, 1]), _pack(hs[:, :, 2])], axis=2)
        yB = np.concatenate([_pack(ys[:, :, 0]), _pack(ys[:, :, 1])], axis=2)
        in_maps.append({
            "hA": np.ascontiguousarray(hA),
            "hB": np.ascontiguousarray(hB),
            "yB": np.ascontiguousarray(yB),
        })
    return in_maps


def assemble_output(results):
    """Per-core xout [NCH, 128, 2FD] -> full [B, U, A, S, F]."""
    out = np.empty((B, U, A, S, F), np.float32)
    for c in range(NCORES):
        xo = np.asarray(results[c]["xout"])
        x0 = _unpack(xo[:, :, :FD]).reshape(BPC, U, S, F)
        x1 = _unpack(xo[:, :, FD:]).reshape(BPC, U, S, F)
        out[c * BPC:(c + 1) * BPC, :, 0] = x0
        out[c * BPC:(c + 1) * BPC, :, 1] = x1
    return out


def kernel(y, h, precoding_ind):
    global LAST_RESULTS
    in_maps = make_in_maps(y, h, precoding_ind)
    nc = _build_nc()
    res = run_bass_kernel_spmd(nc, in_maps, list(range(NCORES)), trace=TRACE)
    LAST_RESULTS = res
    return assemble_output(res.results)



# revision 4
# speedup vs baseline: 1.1258x; 1.1258x over previous
"""Block-diagonal 2x2 equalizer kernel for Trainium2 (8 NeuronCores), v2.

Per point (b, u, s, f) solves the 2x2 system M x = v by Cramer's rule:
    det = m00*m11 - m01*m10
    x0  = (m11*v0 - m01*v1) / det
    x1  = (m00*v1 - m10*v0) / det

Mixed precision (validated vs reference: rel err 3.3e-4, gate 2e-2):
  - det chain MUST be fp32: the data has near-singular blocks
    (min |det| = 1.5e-4 while |p0|,|p1| ~ 10); fp16 m-quantization alone
    perturbs det by ~1e-2 -> div-by-zero / garbage at those points.
  - numerators/output are fp16: error there is relative-in-r, bounded by
    ~|x|*1e-3 even at singular points, and the gate is absolute
    (err.max()/|x|.max()).

Per-chunk engine program (F = free cols per chunk, 2 chunks of 896):
  DVE (9 ops, fp16 ops run in 2x_1P mode, 2 elem/lane/cyc):
    PM0  p0 = m00*m11              [F]  fp32
    QA   (q0,q1) = [m11|m01]*[v0|v1] [2F] fp16   <- paired mul trick
    PM1  p1 = m01*m10              [F]  fp32
    DSUB det = p0 - p1             [F]  fp32
    QB   (q3,q2) = [m10|m00]*[v0|v1] [2F] fp16   <- shares the v window
    RS0  r0 = q0 - q1              [F]  fp16
    RS1  r1 = q2 - q3              [F]  fp16
    FX0  x0 = r0*rdet              [F]  fp16
    FX1  x1 = r1*rdet              [F]  fp16
  ACT: rdet = Reciprocal(det) fp32->fp16 (HW spline, 2.2e-5 rel err),
       issued right after DSUB so it hides under QB/RS0/RS1.
  Stores: [x0|x1] split by partition halves on scalar+tensor engines
       (parallel DMA descriptor generation; desc-gen is ~18-29ns/row).

DMA: host packs each chunk into contiguous strips so every transfer has
7168B descriptors; input sub-DMAs are interleaved (A-half, B-half per
chunk) so DVE starts after the first 917KB.

Raw Bass (no TileContext): standalone wait_ge only (walrus allows one
sync-wait per instruction); every SBUF buffer written exactly once.
"""

from contextlib import ExitStack

import numpy as np

import concourse.bass as bass
import concourse.mybir as mybir
from concourse.bass_utils import run_bass_kernel_spmd

# Problem shapes (hardcoded per contract)
B, U, A, NTX, T, S, F = 16, 4, 2, 1, 8, 14, 2048
SF = S * F               # 28672
NCORES = 8
BPC = B // NCORES        # 2 batches per core
QW = 448                 # inner width: SF = 64 * 448
ROWS = SF // QW          # 64 rows -> partition p = b*64 + row
COLS = U * QW            # 1792 free columns per plane
NCH = 2                  # chunks along the free axis
FD = COLS // NCH         # 896 free cols per chunk

TRACE = False
LAST_RESULTS = None


def _grid(x):
    """[BPC, U, SF] -> [128, COLS]; p = b*64 + sf//QW, col = u*QW + sf%QW."""
    return np.ascontiguousarray(
        x.reshape(BPC, U, ROWS, QW).transpose(0, 2, 1, 3).reshape(BPC * ROWS, COLS)
    )


def _ungrid(t):
    """Inverse of _grid: [128, COLS] -> [BPC, U, SF]."""
    return t.reshape(BPC, ROWS, U, QW).transpose(0, 2, 1, 3).reshape(BPC, U, SF)


def _build_nc():
    f32 = mybir.dt.float32
    f16 = mybir.dt.float16
    nc = bass.Bass("TRN2")
    # hA[k,0] = [m00|m11] f32, hA[k,1] = [m01|m10] f32
    hA = nc.dram_tensor("hA", [NCH, 2, 128, 2 * FD], f32, kind="ExternalInput")
    # hBa[k] = [m11|m01|v0|v1] f16, hBb[k] = [m10|m00] f16
    hBa = nc.dram_tensor("hBa", [NCH, 128, 4 * FD], f16, kind="ExternalInput")
    hBb = nc.dram_tensor("hBb", [NCH, 128, 2 * FD], f16, kind="ExternalInput")
    xout = nc.dram_tensor("xout", [NCH, 128, 2 * FD], f16, kind="ExternalOutput")

    with ExitStack() as ctx:
        tA = [ctx.enter_context(nc.sbuf_tensor(f"tA{k}", [128, 4 * FD], f32)) for k in range(NCH)]
        tB = [ctx.enter_context(nc.sbuf_tensor(f"tB{k}", [128, 6 * FD], f16)) for k in range(NCH)]
        tP = [ctx.enter_context(nc.sbuf_tensor(f"tP{k}", [128, 2 * FD], f32)) for k in range(NCH)]
        tQ = [ctx.enter_context(nc.sbuf_tensor(f"tQ{k}", [128, 4 * FD], f16)) for k in range(NCH)]
        tD = [ctx.enter_context(nc.sbuf_tensor(f"tD{k}", [128, FD], f32)) for k in range(NCH)]
        tW = [ctx.enter_context(nc.sbuf_tensor(f"tW{k}", [128, FD], f16)) for k in range(NCH)]
        tR = [ctx.enter_context(nc.sbuf_tensor(f"tR{k}", [128, 2 * FD], f16)) for k in range(NCH)]
        tX = [ctx.enter_context(nc.sbuf_tensor(f"tX{k}", [128, 2 * FD], f16)) for k in range(NCH)]
        semA0 = [ctx.enter_context(nc.semaphore(f"semA0_{k}")) for k in range(NCH)]
        semA1 = [ctx.enter_context(nc.semaphore(f"semA1_{k}")) for k in range(NCH)]
        semBa = [ctx.enter_context(nc.semaphore(f"semBa_{k}")) for k in range(NCH)]
        semBb = [ctx.enter_context(nc.semaphore(f"semBb_{k}")) for k in range(NCH)]
        dve_sem = ctx.enter_context(nc.semaphore("dve_sem"))
        act_sem = ctx.enter_context(nc.semaphore("act_sem"))
        semO = ctx.enter_context(nc.semaphore("semO"))

        with nc.Block() as block:

            @block.sync
            def _(sync):
                # interleave fp32/fp16 halves so DVE's dependency order
                # matches arrival order
                for k in range(NCH):
                    sync.dma_start(out=tA[k][:, : 2 * FD], in_=hA[k, 0]).then_inc(semA0[k], 16)
                    sync.dma_start(out=tB[k][:, : 4 * FD], in_=hBa[k]).then_inc(semBa[k], 16)
                    sync.dma_start(out=tA[k][:, 2 * FD :], in_=hA[k, 1]).then_inc(semA1[k], 16)
                    sync.dma_start(out=tB[k][:, 4 * FD :], in_=hBb[k]).then_inc(semBb[k], 16)
                sync.wait_ge(semO, 16 * 2 * NCH)  # 2 stores per chunk, +16 each

            # dve_sem counts: chunk k ops are 9k+1 .. 9k+9
            @block.vector
            def _(vector):
                for k in range(NCH):
                    a, b, q, p, r, x = tA[k], tB[k], tQ[k], tP[k], tR[k], tX[k]
                    vector.wait_ge(semA0[k], 16)
                    vector.tensor_mul(p[:, :FD], a[:, :FD], a[:, FD : 2 * FD]).then_inc(dve_sem, 1)   # PM0
                    vector.wait_ge(semBa[k], 16)
                    vector.tensor_mul(q[:, : 2 * FD], b[:, : 2 * FD], b[:, 2 * FD : 4 * FD]).then_inc(dve_sem, 1)  # QA
                    vector.wait_ge(semA1[k], 16)
                    vector.tensor_mul(p[:, FD:], a[:, 2 * FD : 3 * FD], a[:, 3 * FD :]).then_inc(dve_sem, 1)       # PM1
                    vector.tensor_sub(tD[k][:], p[:, :FD], p[:, FD:]).then_inc(dve_sem, 1)            # DSUB (ACT consumes)
                    vector.wait_ge(semBb[k], 16)
                    vector.tensor_mul(q[:, 2 * FD :], b[:, 4 * FD :], b[:, 2 * FD : 4 * FD]).then_inc(dve_sem, 1)  # QB
                    vector.tensor_sub(r[:, :FD], q[:, :FD], q[:, FD : 2 * FD]).then_inc(dve_sem, 1)   # RS0
                    vector.tensor_sub(r[:, FD:], q[:, 3 * FD :], q[:, 2 * FD : 3 * FD]).then_inc(dve_sem, 1)       # RS1
                    vector.wait_ge(act_sem, k + 1)
                    vector.tensor_mul(x[:, :FD], r[:, :FD], tW[k][:]).then_inc(dve_sem, 1)            # FX0
                    vector.tensor_mul(x[:, FD:], r[:, FD:], tW[k][:]).then_inc(dve_sem, 1)            # FX1

            @block.scalar
            def _(scalar):
                f32i = mybir.dt.float32
                for k in range(NCH):
                    scalar.wait_ge(dve_sem, 9 * k + 4)
                    scalar.add_instruction(
                        mybir.InstActivation(
                            name=nc.get_next_instruction_name(),
                            func=mybir.ActivationFunctionType.Reciprocal,
                            ins=[
                                scalar.lower_ap(tD[k][:]),
                                mybir.ImmediateValue(dtype=f32i, value=0.0),
                                mybir.ImmediateValue(dtype=f32i, value=1.0),
                                mybir.ImmediateValue(dtype=f32i, value=0.0),
                            ],
                            outs=[scalar.lower_ap(tW[k][:])],
                        )
                    ).then_inc(act_sem, 1)
                for k in range(NCH):
                    scalar.wait_ge(dve_sem, 9 * k + 9)
                    scalar.dma_start(out=xout[k, 0:64, :], in_=tX[k][0:64, :]).then_inc(semO, 16)

            @block.gpsimd
            def _(gpsimd):
                for k in range(NCH):
                    gpsimd.wait_ge(dve_sem, 9 * k + 9)
                    gpsimd.dma_start(out=xout[k, 64:128, :], in_=tX[k][64:128, :]).then_inc(semO, 16)

    return nc


def make_in_maps(y, h, precoding_ind):
    """Host-side gather + dtype cast + strip packing. Returns per-core maps."""
    y = np.asarray(y, dtype=np.float32)
    h = np.asarray(h, dtype=np.float32)
    pi = np.asarray(precoding_ind).astype(np.int64)

    hg = h[:, pi[0]]                                     # [B, U, A, NTX, T, S, F]
    # msel[b, u, i, j] = hg[b, u, i, 0, 2u+j]
    msel = np.stack(
        [hg[:, u, :, 0, 2 * u : 2 * u + 2] for u in range(U)], axis=1
    )                                                    # [B, U, A(i), 2(j), S, F]
    msel = np.ascontiguousarray(msel).reshape(B, U, 2, 2, SF)
    yr = np.ascontiguousarray(y).reshape(B, U, A, SF)

    in_maps = []
    for c in range(NCORES):
        b0 = c * BPC
        ms = msel[b0 : b0 + BPC]                         # [BPC, U, 2, 2, SF]
        ys = yr[b0 : b0 + BPC]                           # [BPC, U, A, SF]
        g32 = {
            name: _grid(ms[:, :, i, j])
            for name, (i, j) in {"m00": (0, 0), "m01": (0, 1), "m10": (1, 0), "m11": (1, 1)}.items()
        }
        v0g, v1g = _grid(ys[:, :, 0]), _grid(ys[:, :, 1])
        g16 = {n: a.astype(np.float16) for n, a in g32.items()}
        v0h, v1h = v0g.astype(np.float16), v1g.astype(np.float16)

        hA = np.empty((NCH, 2, 128, 2 * FD), np.float32)
        hBa = np.empty((NCH, 128, 4 * FD), np.float16)
        hBb = np.empty((NCH, 128, 2 * FD), np.float16)
        for k in range(NCH):
            s = slice(k * FD, (k + 1) * FD)
            hA[k, 0] = np.concatenate([g32["m00"][:, s], g32["m11"][:, s]], axis=1)
            hA[k, 1] = np.concatenate([g32["m01"][:, s], g32["m10"][:, s]], axis=1)
            hBa[k] = np.concatenate(
                [g16["m11"][:, s], g16["m01"][:, s], v0h[:, s], v1h[:, s]], axis=1
            )
            hBb[k] = np.concatenate([g16["m10"][:, s], g16["m00"][:, s]], axis=1)
        in_maps.append({
            "hA": np.ascontiguousarray(hA),
            "hBa": np.ascontiguousarray(hBa),
            "hBb": np.ascontiguousarray(hBb),
        })
    return in_maps


def assemble_output(results):
    """Per-core xout [NCH, 128, 2FD] f16 -> full [B, U, A, S, F] f32."""
    out = np.empty((B, U, A, S, F), np.float32)
    for c in range(NCORES):
        xo = np.asarray(results[c]["xout"]).astype(np.float32)
        x0 = np.empty((128, COLS), np.float32)
        x1 = np.empty((128, COLS), np.float32)
        for k in range(NCH):
            s = slice(k * FD, (k + 1) * FD)
            x0[:, s] = xo[k, :, :FD]
            x1[:, s] = xo[k, :, FD:]
        out[c * BPC : (c + 1) * BPC, :, 0] = _ungrid(x0).reshape(BPC, U, S, F)
        out[c * BPC : (c + 1) * BPC, :, 1] = _ungrid(x1).reshape(BPC, U, S, F)
    return out


def kernel(y, h, precoding_ind):
    global LAST_RESULTS
    in_maps = make_in_maps(y, h, precoding_ind)
    nc = _build_nc()
    res = run_bass_kernel_spmd(nc, in_maps, list(range(NCORES)), trace=TRACE)
    LAST_RESULTS = res
    return assemble_output(res.results)
